# revision 12
# baseline (speedup 1.0000x reference)
"""Trainium2 Bass kernel for nn_DeepSignatureModel (depth-2 signature model).

Self-contained: hardcodes shapes from the problem spec.
  x: (64, 1024, 5) f32, lengths: (64,) int64  ->  out: (64, 32) f32

Strategy (pure data parallel, 8 batch elements per core):
  - Depth-2 signature stream == cumsum of rank-1 outer products:
        a1[t] = h[t];  a2[t] = sum_{s<=t} m[s] (x) dx[s],  m = (h[t]+h[t-1])/2
  - Conv over the signature stream is restructured so the 484-wide cumsum
    becomes a 64-wide cumsum (u-restructure):
        y_a2[t] = cumsum(u0)[t] + u1[t+1] + u2[t+2] + u3[t+3],
        uk = Vk @ g,  Vk = sum_{j>=k} W_j(a2 part),  g[ij,t] = m'_i dx_j  (m' = 2m)
  - g built channel-major via PE expansion matmuls (row-replication with 0/1
    selection matrices) + one DVE multiply.  The standalone dx-part of uk
    (the a1 channels of the conv) is folded into the same chunks: each of the
    4 row-chunks is widened from 121 to 128 rows, the extra rows carrying raw
    dx channels (ones-row trick in the m-expansion), so no separate vh
    matmuls are needed.  u0/u1 and u2/u3 are each fused into single 128-out
    matmuls over the same rhs, with +1/+2/+3 time shifts at the shifted-read
    adds.
  - become_constant handled by masking dx2; signature2 is two matmul
    contractions over time (time-major h2 via data-stationary matmuls).
  - The batch loop is software-pipelined: conv stack of batch b+1 issues
    before the signature path of batch b, and the signature2 reduction of
    batch b-1 fills the PE while batch b's DVE scan chain completes.
  - Final tiny linear (32x272 per batch) applied on host.
"""

import numpy as np

import concourse.bass as bass
import concourse.bacc as bacc
import concourse.mybir as mybir
import concourse.tile as tile
from concourse.bass_utils import run_bass_kernel_spmd

# ---- problem constants ----
K = 4
B, S, CIN = 64, 1024, 5
H1, H2 = 64, 16
OUT = 32
C1 = 22                    # channels entering signature1
C1R = 23                   # + ones row for the dx-fold
CSQ = C1 * C1              # 484
L1 = S - K + 1             # 1021
L2 = L1 - K + 1            # 1018
NB = 8                     # batches per core
NCORES = 8
T = 1024                   # padded time axis
NCH = 4                    # g row-chunks
CHW = 121                  # g rows per chunk (484 = 4*121)
CW = 128                   # widened chunk rows (121 g + up to 7 dx)
# dx channels folded into chunk c (4 chunks cover all 22)
EXC = [list(range(7 * c, 7 * c + 7)) for c in range(3)] + [[21]]
# device h channel order: conv-out(16), x(5), time(1); PERM[new_row] = orig_chan
PERM = list(range(6, 22)) + list(range(0, 5)) + [5]
F32 = mybir.dt.float32
F32R = mybir.dt.float32r
BF16 = mybir.dt.bfloat16

_COMPILED = None
TRACE = False
LAST = None


def build_program():
    nc = bacc.Bacc()

    def inp(name, shape, dt_=F32):
        return nc.declare_dram_parameter(name, list(shape), dt_, isOutput=False)

    # per-core data
    xs_d = inp("xs", (NB, 20, T), F32R)    # im2col of x (k*5+c, t) padded
    mask_d = inp("masktm", (128, NB * 8), F32R)
    # shared constants
    trow_d = inp("trow", (1, T))
    w0p_d = inp("w0p", (20, H1), F32R)
    b0_d = inp("b0c", (H1, 1))
    w1a_d = inp("w1a", (H1, H1), F32R)
    b1_d = inp("b1c", (H1, 1))
    w2a_d = inp("w2a", (H1 + 1, H2), F32R)  # ones-row bias fold
    ri_d = inp("ri", (C1R, NCH * CW), BF16)
    rj_d = inp("rj", (C1R, NCH * CW), BF16)
    v01_d = inp("v01", (CW, 4 * 128), F32R)  # [V0^T | V1^T] per chunk (halved)
    v23_d = inp("v23", (CW, 4 * 128), F32R)  # [V2^T | V3^T] per chunk (halved)
    ba0_d = inp("ba0c", (H1, 1))           # a2 conv0 bias (ACT relu bias)
    w1b_d = inp("w1b", (H1, H1), F32R)
    b1b_d = inp("b1bc", (H1, 1))
    w2b_d = inp("w2b", (H1 + 1, H2), F32R)  # ones-row bias fold
    # outputs
    f2o_d = nc.declare_dram_parameter("f2o", [H2, NB * H2], F32, isOutput=True)
    f1o_d = nc.declare_dram_parameter("f1o", [1, T], F32, isOutput=True)

    with tile.TileContext(nc) as tc:
        with (
            tc.tile_pool(name="const", bufs=1) as cpool,
            tc.tile_pool(name="xin", bufs=2) as xpool,
            tc.tile_pool(name="gbuf", bufs=2) as gpool,
            tc.tile_pool(name="ybuf", bufs=2) as ypool,
            tc.tile_pool(name="alls", bufs=1) as spool,
            tc.tile_pool(name="slots", bufs=1) as slpool,
            tc.tile_pool(name="cvp", bufs=2, space="PSUM") as cv_ps,
            tc.tile_pool(name="expp", bufs=2, space="PSUM") as exp_ps,
            tc.tile_pool(name="up", bufs=1, space="PSUM") as u_ps,
            tc.tile_pool(name="cp", bufs=1, space="PSUM") as c_ps,
        ):
            def cload(dram, shape):
                t_ = cpool.tile(list(shape), dram.dtype, tag=dram.name)
                nc.sync.dma_start(out=t_[:], in_=dram.ap())
                return t_

            Relu = mybir.ActivationFunctionType.Relu
            Copy = mybir.ActivationFunctionType.Copy
            ADD = mybir.AluOpType.add
            SUB = mybir.AluOpType.subtract
            MUL = mybir.AluOpType.mult

            def mm(out, lhsT, rhs, start, stop=None, tile_position=None):
                if stop is None:
                    stop = start
                nc.tensor.matmul(out, lhsT, rhs, start=start,
                                 stop=stop, tile_position=tile_position)

            # ---- critical-path constants first (conv stack of batch 0) ----
            w0p = cload(w0p_d, (20, H1))
            b0c = cload(b0_d, (H1, 1))
            w1a = cload(w1a_d, (H1, H1))
            b1c = cload(b1_d, (H1, 1))
            w2a = cload(w2a_d, (H1 + 1, H2))

            # ---- persistent double-buffered tiles with one-time presets ----
            def mkslots(shape, dt_, tag, n=2):
                return [slpool.tile(list(shape), dt_, tag=f"{tag}{i}", name=f"{tag}{i}") for i in range(n)]

            r0s = mkslots((H1 + 1, T), F32R, "r0")
            r1s = mkslots((H1 + 1, T), F32R, "r1")
            ys = mkslots((H1 + 1, T), F32R, "y")
            r2s = mkslots((H1 + 1, T), F32R, "r2")
            hs = mkslots((C1, T), F32, "h")
            mreps = mkslots((C1R, T), BF16, "mrep")
            dxreps = mkslots((C1R, T), BF16, "dxrep")
            gbigs = mkslots((CW, 4 * T), F32R, "g")

            zeros = cpool.tile([H1, T], F32, tag="zeros")
            nc.vector.memset(zeros[:], 0.0)
            onesrow = cpool.tile([1, T], F32, tag="onesrow")
            nc.vector.memset(onesrow[:], 1.0)
            onesrow_bf = cpool.tile([1, T], BF16, tag="onesrow_bf")
            nc.vector.memset(onesrow_bf[:], 1.0)
            zrow_bf = cpool.tile([1, T], BF16, tag="zrow_bf")
            nc.vector.memset(zrow_bf[:], 0.0)
            onescol = cpool.tile([128, 1], F32, tag="onescol")
            nc.vector.memset(onescol[:], 1.0)

            # time row of h (data) via DMA; other presets are memsets except
            # rows at unaligned partition offsets (DMA from const rows)
            for t_ in hs:
                nc.sync.dma_start(out=t_[21:22, :], in_=trow_d.ap())
                nc.vector.memset(t_[:, L1:T], 0.0)
            for i, t_ in enumerate(r0s + r1s):
                eng = nc.vector
                eng.memset(t_[H1 : H1 + 1, :].bitcast(F32), 1.0)
                eng.memset(t_[0:H1, L1:T].bitcast(F32), 0.0)
            for i, t_ in enumerate(ys + r2s):
                eng = nc.vector
                eng.memset(t_[H1 : H1 + 1, :].bitcast(F32), 1.0)
                eng.memset(t_[0:H1, L2:T].bitcast(F32), 0.0)
            for i, t_ in enumerate(mreps):
                eng = nc.vector
                eng.memset(t_[0:C1R, L1:T], 0.0)
                nc.sync.dma_start(
                    out=t_[C1:C1R, 0:L1], in_=onesrow_bf[0:1, 0:L1]
                )   # ones row for dx-fold
            for i, t_ in enumerate(dxreps):
                eng = nc.vector
                eng.memset(t_[0:C1R, L1:T], 0.0)
                nc.sync.dma_start(
                    out=t_[C1:C1R, 0:L1], in_=zrow_bf[0:1, 0:L1]
                )

            # all-batch tiles (column block per batch)
            h2all = spool.tile([128, NB * 128], F32, tag="h2all")
            h2sh = spool.tile([128, NB * 128], F32, tag="h2sh")
            d2f = spool.tile([128, NB * 128], F32, tag="d2f")
            d2 = spool.tile([128, NB * 128], F32R, tag="d2")
            m2p = spool.tile([128, NB * 128], F32R, tag="m2p")
            f2sb = spool.tile([H2, NB * H2], F32, tag="f2sb")
            f1sb = spool.tile([1, T], F32, tag="f1sb")
            nc.vector.memset(h2sh[0:1, :], 0.0)

            def stage_a(b):
                """augment1 conv stack + h assembly + m'/dx (pool)."""
                xsb = xpool.tile([20, T], F32R, tag="xsb")
                nc.sync.dma_start(out=xsb[:], in_=xs_d.ap()[b])

                r0 = r0s[b % 2]
                r1 = r1s[b % 2]
                h = hs[b % 2]
                for h0 in (0, 512):
                    ps0 = cv_ps.tile([H1, 512], F32, tag="cv")
                    mm(ps0[:], w0p[:], xsb[:, h0 : h0 + 512], True)
                    n = min(512, L1 - h0)
                    nc.scalar.activation(
                        r0[0:H1, h0 : h0 + n], ps0[:, 0:n], Relu, bias=b0c[:]
                    )
                for h0 in (0, 512):
                    ps1 = cv_ps.tile([H1, 512], F32, tag="cv")
                    mm(ps1[:], w1a[:], r0[0:H1, h0 : h0 + 512], True)
                    n = min(512, L1 - h0)
                    nc.scalar.activation(
                        r1[0:H1, h0 : h0 + n], ps1[:, 0:n], Relu, bias=b1c[:]
                    )
                for h0 in (0, 512):
                    ps2 = cv_ps.tile([H2, 512], F32, tag="cv")
                    mm(ps2[:], w2a[:], r1[:, h0 : h0 + 512], True)
                    n = min(512, L1 - h0)
                    nc.scalar.activation(
                        h[0:H2, h0 : h0 + n], ps2[:, 0:n], Copy
                    )

                # ---- assemble h (22, 1021): conv out, x[t+3,:], time ----
                nc.sync.dma_start(
                    out=h[16:21, 0:L1], in_=xs_d.ap()[b, 15:20, 0:L1].bitcast(F32)
                )

                # ---- m' = h[t]+h[t-1], dx = h[t]-h[t-1] (pool engine) ----
                mrep = mreps[b % 2]
                dxrep = dxreps[b % 2]
                nc.gpsimd.tensor_tensor(
                    mrep[0:C1, 1:L1], h[:, 1:L1], h[:, 0 : L1 - 1], ADD
                )
                nc.gpsimd.tensor_tensor(
                    dxrep[0:C1, 1:L1], h[:, 1:L1], h[:, 0 : L1 - 1], SUB
                )
                nc.gpsimd.tensor_copy(mrep[0:C1, 0:1], h[:, 0:1])
                nc.gpsimd.tensor_copy(dxrep[0:C1, 0:1], h[:, 0:1])

            def sig2_prep(b, c0, c1):
                """shift/diff/mask/sum of the h2 block cols [c0:c1)."""
                col = 128 * b
                nc.sync.dma_start(
                    out=h2sh[1:128, col + c0 : col + c1],
                    in_=h2all[0:127, col + c0 : col + c1],
                )
                j0, j1 = c0 // 16, c1 // 16
                jlo = max(j0, 1)
                sh_view = h2sh[0:1, col + 16 * jlo : col + 16 * j1].rearrange(
                    "p (j c) -> p j c", j=j1 - jlo
                )
                src_view = h2all[
                    127:128, col + 16 * (jlo - 1) : col + 16 * (j1 - 1)
                ].rearrange("p (j c) -> p j c", j=j1 - jlo)
                nc.sync.dma_start(out=sh_view, in_=src_view)

                eng_s = nc.vector if b == NB - 1 else nc.gpsimd
                eng_s.tensor_tensor(
                    d2f[:, col + c0 : col + c1],
                    h2all[:, col + c0 : col + c1],
                    h2sh[:, col + c0 : col + c1],
                    SUB,
                )
                mview = maskt[:, 8 * b + j0 : 8 * b + j1].unsqueeze(2)
                nc.vector.tensor_tensor(
                    d2[:, col + c0 : col + c1].rearrange(
                        "p (j c) -> p j c", j=j1 - j0
                    ),
                    d2f[:, col + c0 : col + c1].rearrange(
                        "p (j c) -> p j c", j=j1 - j0
                    ),
                    mview.broadcast_to((128, j1 - j0, H2)),
                    MUL,
                )
                eng_s.tensor_tensor(
                    m2p[:, col + c0 : col + c1],
                    h2all[:, col + c0 : col + c1],
                    h2sh[:, col + c0 : col + c1],
                    ADD,
                )

            def stage_c_pe(b):
                """signature2 reductions for batch b (PE + tiny copies)."""
                col = 128 * b
                psF = cv_ps.tile([H2, H2], F32, tag="cv")
                for j in range(8):
                    cj = col + 16 * j
                    mm(
                        psF[:],
                        m2p[:, cj : cj + 16],
                        d2[:, cj : cj + 16],
                        j == 0,
                        j == 7,
                    )
                nc.scalar.activation(f2sb[:, H2 * b : H2 * (b + 1)], psF[:], Copy)
                psF1 = cv_ps.tile([1, 128], F32, tag="cv")
                mm(psF1[:], onescol[:].bitcast(F32R), d2[:, col : col + 128], True)
                nc.scalar.activation(f1sb[:, col : col + 128], psF1[:], Copy)

            def stage_b(b):
                """expansion, u matmuls, scan, y, augment2, h2, sig2 prep."""
                mrep = mreps[b % 2]
                dxrep = dxreps[b % 2]
                gbig = gbigs[b % 2]

                # ---- expansions + g = m'_exp * dx_exp (channel-major) ----
                mexp = gpool.tile([CW, 4 * T], BF16, tag="mexp")
                for phase in range(8):
                    h0 = 512 * (phase % 2)
                    c = phase // 2
                    pool_p = exp_ps if phase % 2 == 0 else cv_ps
                    mps = pool_p.tile([CW, 512], F32, tag="cv" if phase % 2 else "exp")
                    mm(
                        mps[:],
                        ri[:, CW * c : CW * (c + 1)],
                        mrep[:, h0 : h0 + 512],
                        True,
                    )
                    mview = mexp[:, T * c + h0 : T * c + h0 + 512]
                    if phase % 4 in (0, 2) or (b == NB - 1 and phase % 4 == 3):
                        nc.scalar.activation(mview, mps[:], Copy)
                    else:
                        nc.vector.tensor_copy(mview, mps[:])
                    dps = pool_p.tile([CW, 512], F32, tag="cv" if phase % 2 else "exp")
                    mm(
                        dps[:],
                        rj[:, CW * c : CW * (c + 1)],
                        dxrep[:, h0 : h0 + 512],
                        True,
                    )
                    gview = gbig[:, T * c + h0 : T * c + h0 + 512]
                    nc.vector.tensor_tensor(gview, dps[:], mview, MUL)

                # ---- u0/u1 matmuls (fused 128-out, same rhs) ----
                psA = u_ps.tile([128, T], F32, tag="uA")   # rows 0:64 u0, 64:128 u1
                for h0 in (0, 512):
                    for c in range(4):
                        mm(
                            psA[:, h0 : h0 + 512],
                            v01[:, 128 * c : 128 * (c + 1)],
                            gbig[:, T * c + h0 : T * c + h0 + 512],
                            c == 0,
                            c == 3,
                        )
                # ---- cumsum(u0) + u1[t+1] ----
                scn = ypool.tile([H1, T], F32, tag="scn")
                nc.vector.tensor_tensor_scan(
                    scn[:, 0:L1], psA[0:H1, 0:L1], zeros[:, 0:L1], 0.0, ADD, ADD
                )
                t1 = ypool.tile([H1, T], F32, tag="t1")
                nc.vector.tensor_tensor(
                    t1[:, 0:L2], scn[:, 0:L2], psA[H1:128, 1 : 1 + L2], ADD
                )

                # ---- u2/u3 matmuls (fused 128-out, same rhs) ----
                psC = c_ps.tile([128, T], F32, tag="uC")   # rows 0:64 u2, 64:128 u3
                for h0 in (0, 512):
                    for c in range(4):
                        mm(
                            psC[:, h0 : h0 + 512],
                            v23[:, 128 * c : 128 * (c + 1)],
                            gbig[:, T * c + h0 : T * c + h0 + 512],
                            c == 0,
                            c == 3,
                        )

                # ---- y = relu(t1 + u2[t+2] + u3[t+3] + bias) ----
                y = ys[b % 2]
                for h0 in (0, 512):
                    n = min(512, L2 - h0)
                    t2 = ypool.tile([H1, 512], F32, tag="t2")
                    nc.vector.tensor_tensor(
                        t2[:, 0:n],
                        t1[:, h0 : h0 + n],
                        psC[0:H1, h0 + 2 : h0 + 2 + n],
                        ADD,
                    )
                    t3 = ypool.tile([H1, 512], F32, tag="t3")
                    nc.vector.tensor_tensor(
                        t3[:, 0:n],
                        t2[:, 0:n],
                        psC[H1:128, h0 + 3 : h0 + 3 + n],
                        ADD,
                    )
                    nc.scalar.activation(
                        y[0:H1, h0 : h0 + n], t3[:, 0:n], Relu, bias=ba0c[:]
                    )

                # ---- fill PE with previous batch's sig2 reduction ----
                if b >= 1:
                    stage_c_pe(b - 1)

                # ---- augment2 pointwise convs ----
                r2 = r2s[b % 2]
                for h0 in (0, 512):
                    psY = cv_ps.tile([H1, 512], F32, tag="cv")
                    mm(psY[:], w1b[:], y[0:H1, h0 : h0 + 512], True)
                    n = min(512, L2 - h0)
                    nc.scalar.activation(
                        r2[0:H1, h0 : h0 + n], psY[:, 0:n], Relu, bias=b1bc[:]
                    )

                # conv2_2 data-stationary -> time-major h2 (t, ch) per block;
                # for the last batch, halve blocks so the sig2 chain pipelines
                col = 128 * b
                halves = ((0, 8),) if b < NB - 1 else ((0, 4), (4, 8))
                psH = cv_ps.tile([128, 128], F32, tag="cv")
                for (ja, jb) in halves:
                    for j in range(ja, jb):
                        mm(
                            psH[:, 16 * j : 16 * j + 16],
                            r2[:, 128 * j : 128 * j + 128],
                            w2b[:],
                            True,
                        )
                    nc.scalar.activation(
                        h2all[:, col + 16 * ja : col + 16 * jb],
                        psH[:, 16 * ja : 16 * jb],
                        Copy,
                    )
                    sig2_prep(b, 16 * ja, 16 * jb)

            # ---- head: presets done above; run the pipelined batch loop ----
            stage_a(0)

            # remaining constants (off the batch-0 critical path)
            ri = cload(ri_d, (C1R, NCH * CW))
            rj = cload(rj_d, (C1R, NCH * CW))
            v01 = cload(v01_d, (CW, 4 * 128))
            v23 = cload(v23_d, (CW, 4 * 128))
            ba0c = cload(ba0_d, (H1, 1))
            w1b = cload(w1b_d, (H1, H1))
            b1bc = cload(b1b_d, (H1, 1))
            w2b = cload(w2b_d, (H1 + 1, H2))
            maskt = cload(mask_d, (128, NB * 8))

            for b in range(NB):
                if b + 1 < NB:
                    stage_a(b + 1)
                stage_b(b)
            stage_c_pe(NB - 1)

            nc.sync.dma_start(out=f2o_d.ap(), in_=f2sb[:])
            nc.sync.dma_start(out=f1o_d.ap(), in_=f1sb[:])

    return nc


def _prep_host(x, lengths):
    """host-side preprocessing -> per-core input maps + host weights"""
    x = np.ascontiguousarray(x, dtype=np.float32)
    lengths = np.asarray(lengths).astype(np.int64)

    xs = np.zeros((B, 20, T), np.float32)
    for k in range(K):
        xs[:, 5 * k : 5 * k + 5, 0:L1] = x[:, k : k + L1, :].transpose(0, 2, 1)

    adj = (lengths - 2 * K + 2).astype(np.int64)  # (64,)
    # mask in time-major packed layout: mask[p, b*8+j] = 1 if (128j+p) < adj_b (and < L2)
    tgrid = (np.arange(8)[None, :] * 128 + np.arange(128)[:, None])  # (128, 8)
    masks = []
    for core in range(NCORES):
        mcols = np.zeros((128, NB * 8), np.float32)
        for b in range(NB):
            a = min(int(adj[core * NB + b]), L2)
            mcols[:, b * 8 : (b + 1) * 8] = (tgrid < a).astype(np.float32)
        masks.append(mcols)
    return xs, masks, adj


def round_f32r(a):
    """round-to-nearest-even to 11-bit mantissa (fp32r)"""
    u = np.ascontiguousarray(a, np.float32).view(np.uint32)
    u = (u + 0x7FF + ((u >> 12) & 1)) & np.uint32(0xFFFFF000)
    return u.view(np.float32)


F32R_KEYS = {"xs", "masktm", "w0p", "w1a", "w2a", "w1b", "w2b", "v01", "v23"}
BF16_KEYS = {"ri", "rj"}


def _prep_weights(inp):
    w = {}
    w["trow"] = np.zeros((1, T), np.float32)
    w["trow"][0, :L1] = np.linspace(0.0, 1.0, L1, dtype=np.float32)

    a1_w0 = inp["a1_w0"]
    w0p = np.zeros((20, H1), np.float32)
    for k in range(K):
        w0p[5 * k : 5 * k + 5, :] = a1_w0[:, :, k].T
    w["w0p"] = w0p
    w["b0c"] = inp["a1_b0"].reshape(H1, 1).astype(np.float32)
    w["w1a"] = inp["a1_w1"][:, :, 0].T.astype(np.float32)
    w["b1c"] = inp["a1_b1"].reshape(H1, 1).astype(np.float32)
    w2a = np.zeros((H1 + 1, H2), np.float32)
    w2a[0:H1] = inp["a1_w2"][:, :, 0].T
    w2a[H1] = inp["a1_b2"]
    w["w2a"] = w2a

    # selection matrices: 4 chunks of 128 cols; cols 0:121 replicate the g
    # rows, cols 121: carry raw dx channels (ones row in ri, channel in rj)
    ri = np.zeros((C1R, NCH * CW), np.float32)
    rj = np.zeros((C1R, NCH * CW), np.float32)
    for c in range(NCH):
        for p in range(CHW):
            g = CHW * c + p
            ri[g // C1, CW * c + p] = 1.0
            rj[g % C1, CW * c + p] = 1.0
        for e, ch in enumerate(EXC[c]):
            ri[C1, CW * c + CHW + e] = 1.0
            rj[ch, CW * c + CHW + e] = 1.0
    w["ri"] = ri
    w["rj"] = rj

    w20 = inp["a2_w0"]  # (64, 506, 4)
    # permute the a2 (484) block to the device h-channel order
    pidx = (np.array(PERM)[:, None] * C1 + np.array(PERM)[None, :]).reshape(-1)
    Wk_a = [w20[:, C1:, k].astype(np.float64)[:, pidx] for k in range(K)]
    V = [None] * 4
    V[3] = Wk_a[3]
    V[2] = Wk_a[2] + V[3]
    V[1] = Wk_a[1] + V[2]
    V[0] = Wk_a[0] + V[1]
    perm = PERM
    Wh = [w20[:, perm, k].T.astype(np.float64) for k in range(K)]  # (22, 64)
    Vh = [None] * 4
    Vh[3] = Wh[3]
    Vh[2] = Wh[2] + Vh[3]
    Vh[1] = Wh[1] + Vh[2]
    Vh[0] = Wh[0] + Vh[1]
    # halve the g part for m' = 2m; dx-fold rows are unhalved
    v01 = np.zeros((CW, 4 * 128), np.float32)
    v23 = np.zeros((CW, 4 * 128), np.float32)
    for c in range(NCH):
        rows = slice(CHW * c, CHW * (c + 1))
        v01[0:CHW, 128 * c : 128 * c + 64] = 0.5 * V[0].T[rows]
        v01[0:CHW, 128 * c + 64 : 128 * c + 128] = 0.5 * V[1].T[rows]
        v23[0:CHW, 128 * c : 128 * c + 64] = 0.5 * V[2].T[rows]
        v23[0:CHW, 128 * c + 64 : 128 * c + 128] = 0.5 * V[3].T[rows]
        for e, ch in enumerate(EXC[c]):
            v01[CHW + e, 128 * c : 128 * c + 64] = Vh[0][ch]
            v01[CHW + e, 128 * c + 64 : 128 * c + 128] = Vh[1][ch]
            v23[CHW + e, 128 * c : 128 * c + 64] = Vh[2][ch]
            v23[CHW + e, 128 * c + 64 : 128 * c + 128] = Vh[3][ch]
    w["v01"] = v01
    w["v23"] = v23

    w["ba0c"] = inp["a2_b0"].reshape(H1, 1).astype(np.float32)
    w["w1b"] = inp["a2_w1"][:, :, 0].T.astype(np.float32)
    w["b1bc"] = inp["a2_b1"].reshape(H1, 1).astype(np.float32)
    w2b = np.zeros((H1 + 1, H2), np.float32)
    w2b[0:H1] = inp["a2_w2"][:, :, 0].T
    w2b[H1] = inp["a2_b2"]
    w["w2b"] = w2b
    np_bf16 = mybir.dt.np(mybir.dt.bfloat16)
    for k in list(w):
        if k in F32R_KEYS:
            w[k] = round_f32r(w[k])
        elif k in BF16_KEYS:
            w[k] = w[k].astype(np_bf16)
    return w


def kernel(**inputs):
    global _COMPILED
    x = np.asarray(inputs["x"], np.float32)
    lengths = np.asarray(inputs["lengths"])

    xs, masks, adj = _prep_host(x, lengths)
    w = _prep_weights({k: np.asarray(v) for k, v in inputs.items()})

    if _COMPILED is None:
        _c = build_program()
        _c.finalize()
        _COMPILED = _c
    nc = _COMPILED

    in_maps = []
    for core in range(NCORES):
        m = {"xs": round_f32r(xs[core * NB : (core + 1) * NB]),
             "masktm": masks[core]}
        m.update(w)
        in_maps.append(m)

    _res = run_bass_kernel_spmd(nc, in_maps, list(range(NCORES)), trace=TRACE)
    globals()["LAST"] = _res
    res = _res.results

    # host: assemble s2 and final linear
    lin_w = np.asarray(inputs["lin_w"], np.float32)
    lin_b = np.asarray(inputs["lin_b"], np.float32)
    out = np.zeros((B, OUT), np.float32)
    for core in range(NCORES):
        f2 = res[core]["f2o"]          # (16, NB*16)
        f1 = res[core]["f1o"][0]       # (T,)
        for b in range(NB):
            gb = core * NB + b
            F2 = 0.5 * f2[:, H2 * b : H2 * (b + 1)]          # (16, 16)
            F1 = f1[128 * b : 128 * (b + 1)].reshape(8, H2).sum(axis=0)
            s2 = np.concatenate([F1, F2.reshape(-1)])
            out[gb] = s2 @ lin_w.T + lin_b
    return out.astype(np.float32)


# revision 13
# speedup vs baseline: 1.1046x; 1.1046x over previous
"""Trainium2 Bass kernel for nn_DeepSignatureModel (depth-2 signature model).

Self-contained: hardcodes shapes from the problem spec.
  x: (64, 1024, 5) f32, lengths: (64,) int64  ->  out: (64, 32) f32

Strategy (pure data parallel, 8 batch elements per core):
  - Depth-2 signature stream == cumsum of rank-1 outer products:
        a1[t] = h[t];  a2[t] = sum_{s<=t} m[s] (x) dx[s],  m = (h[t]+h[t-1])/2
  - Conv over the signature stream is restructured so the 484-wide cumsum
    becomes a 64-wide cumsum (u-restructure):
        y_a2[t] = cumsum(u0)[t] + u1[t+1] + u2[t+2] + u3[t+3],
        uk = Vk @ g,  Vk = sum_{j>=k} W_j(a2 part),  g[ij,t] = m'_i dx_j  (m' = 2m)
  - g built channel-major via PE expansion matmuls (row-replication with 0/1
    selection matrices) + one DVE multiply.  The standalone dx-part of uk
    (the a1 channels of the conv) is folded into the same chunks: each of the
    4 row-chunks is widened from 121 to 128 rows, the extra rows carrying raw
    dx channels (ones-row trick in the m-expansion), so no separate vh
    matmuls are needed.  u0/u1 and u2/u3 are each fused into single 128-out
    matmuls over the same rhs, with +1/+2/+3 time shifts at the shifted-read
    adds.
  - become_constant handled by masking dx2; signature2 is two matmul
    contractions over time (time-major h2 via data-stationary matmuls).
  - The batch loop is software-pipelined: conv stack of batch b+1 issues
    before the signature path of batch b, and the signature2 reduction of
    batch b-1 fills the PE while batch b's DVE scan chain completes.
  - Final tiny linear (32x272 per batch) applied on host.
"""

import numpy as np

import concourse.bass as bass
import concourse.bacc as bacc
import concourse.mybir as mybir
import concourse.tile as tile
from concourse.bass_utils import run_bass_kernel_spmd

# ---- problem constants ----
K = 4
B, S, CIN = 64, 1024, 5
H1, H2 = 64, 16
OUT = 32
C1 = 22                    # channels entering signature1
C1R = 23                   # + ones row for the dx-fold
CSQ = C1 * C1              # 484
L1 = S - K + 1             # 1021
L2 = L1 - K + 1            # 1018
NB = 8                     # batches per core
NCORES = 8
T = 1024                   # padded time axis
NCH = 4                    # g row-chunks
CHW = 121                  # g rows per chunk (484 = 4*121)
CW = 128                   # widened chunk rows (121 g + up to 7 dx)
# dx channels folded into chunk c (4 chunks cover all 22)
EXC = [list(range(7 * c, 7 * c + 7)) for c in range(3)] + [[21]]
# device h channel order: conv-out(16), x(5), time(1); PERM[new_row] = orig_chan
PERM = list(range(6, 22)) + list(range(0, 5)) + [5]
F32 = mybir.dt.float32
F32R = mybir.dt.float32r
BF16 = mybir.dt.bfloat16

_COMPILED = None
TRACE = False
LAST = None


def build_program():
    nc = bacc.Bacc()

    def inp(name, shape, dt_=F32):
        return nc.declare_dram_parameter(name, list(shape), dt_, isOutput=False)

    # per-core data
    xs_d = inp("xs", (NB, 20, T), F32R)    # im2col of x (k*5+c, t) padded
    mask_d = inp("masktm", (128, NB * 8), F32R)
    # shared constants
    trow_d = inp("trow", (1, T))
    w0p_d = inp("w0p", (20, H1), F32R)
    b0_d = inp("b0c", (H1, 1))
    w1a_d = inp("w1a", (H1, H1), F32R)
    b1_d = inp("b1c", (H1, 1))
    w2a_d = inp("w2a", (H1 + 1, H2), F32R)  # ones-row bias fold
    ri_d = inp("ri", (C1R, NCH * CW), BF16)
    rj_d = inp("rj", (C1R, NCH * CW), BF16)
    v01_d = inp("v01", (CW, 4 * 128), F32R)  # [V0^T | V1^T] per chunk (halved)
    v23_d = inp("v23", (CW, 4 * 128), F32R)  # [V2^T | V3^T] per chunk (halved)
    ba0_d = inp("ba0c", (H1, 1))           # a2 conv0 bias (ACT relu bias)
    w1b_d = inp("w1b", (H1, H1), F32R)
    b1b_d = inp("b1bc", (H1, 1))
    w2b_d = inp("w2b", (H1 + 1, H2), F32R)  # ones-row bias fold
    # outputs
    f2o_d = nc.declare_dram_parameter("f2o", [H2, NB * H2], F32, isOutput=True)
    f1o_d = nc.declare_dram_parameter("f1o", [1, T], F32, isOutput=True)

    with tile.TileContext(nc) as tc:
        with (
            tc.tile_pool(name="const", bufs=1) as cpool,
            tc.tile_pool(name="xin", bufs=2) as xpool,
            tc.tile_pool(name="gbuf", bufs=2) as gpool,
            tc.tile_pool(name="ybuf", bufs=2) as ypool,
            tc.tile_pool(name="alls", bufs=1) as spool,
            tc.tile_pool(name="slots", bufs=1) as slpool,
            tc.tile_pool(name="cvp", bufs=2, space="PSUM") as cv_ps,
            tc.tile_pool(name="expp", bufs=2, space="PSUM") as exp_ps,
            tc.tile_pool(name="up", bufs=1, space="PSUM") as u_ps,
            tc.tile_pool(name="cp", bufs=1, space="PSUM") as c_ps,
        ):
            def cload(dram, shape):
                t_ = cpool.tile(list(shape), dram.dtype, tag=dram.name)
                nc.sync.dma_start(out=t_[:], in_=dram.ap())
                return t_

            Relu = mybir.ActivationFunctionType.Relu
            Copy = mybir.ActivationFunctionType.Copy
            ADD = mybir.AluOpType.add
            SUB = mybir.AluOpType.subtract
            MUL = mybir.AluOpType.mult

            def mm(out, lhsT, rhs, start, stop=None, tile_position=None):
                if stop is None:
                    stop = start
                nc.tensor.matmul(out, lhsT, rhs, start=start,
                                 stop=stop, tile_position=tile_position)

            # ---- critical-path constants first (conv stack of batch 0) ----
            w0p = cload(w0p_d, (20, H1))
            b0c = cload(b0_d, (H1, 1))
            w1a = cload(w1a_d, (H1, H1))
            b1c = cload(b1_d, (H1, 1))
            w2a = cload(w2a_d, (H1 + 1, H2))

            # ---- persistent double-buffered tiles with one-time presets ----
            def mkslots(shape, dt_, tag, n=2):
                return [slpool.tile(list(shape), dt_, tag=f"{tag}{i}", name=f"{tag}{i}") for i in range(n)]

            r0s = mkslots((H1 + 1, T), F32R, "r0")
            r1s = mkslots((H1 + 1, T), F32R, "r1")
            ys = mkslots((H1 + 1, T), F32R, "y")
            r2s = mkslots((H1 + 1, T), F32R, "r2")
            hs = mkslots((C1, T), F32, "h")
            mreps = mkslots((C1R, T), BF16, "mrep")
            dxreps = mkslots((C1R, T), BF16, "dxrep")
            gbigs = mkslots((CW, 4 * T), F32R, "g")

            zeros = cpool.tile([H1, T], F32, tag="zeros")
            nc.vector.memset(zeros[:], 0.0)
            onesrow = cpool.tile([1, T], F32, tag="onesrow")
            nc.vector.memset(onesrow[:], 1.0)
            onesrow_bf = cpool.tile([1, T], BF16, tag="onesrow_bf")
            nc.vector.memset(onesrow_bf[:], 1.0)
            zrow_bf = cpool.tile([1, T], BF16, tag="zrow_bf")
            nc.vector.memset(zrow_bf[:], 0.0)
            onescol = cpool.tile([128, 1], F32, tag="onescol")
            nc.vector.memset(onescol[:], 1.0)

            # time row of h (data) via DMA; other presets are memsets except
            # rows at unaligned partition offsets (DMA from const rows)
            for t_ in hs:
                nc.sync.dma_start(out=t_[21:22, :], in_=trow_d.ap())
                nc.vector.memset(t_[:, L1:T], 0.0)
            for i, t_ in enumerate(r0s + r1s):
                eng = nc.vector
                eng.memset(t_[H1 : H1 + 1, :].bitcast(F32), 1.0)
                eng.memset(t_[0:H1, L1:T].bitcast(F32), 0.0)
            for i, t_ in enumerate(ys + r2s):
                eng = nc.vector
                eng.memset(t_[H1 : H1 + 1, :].bitcast(F32), 1.0)
                eng.memset(t_[0:H1, L2:T].bitcast(F32), 0.0)
            for i, t_ in enumerate(mreps):
                eng = nc.vector
                eng.memset(t_[0:C1R, L1:T], 0.0)
                nc.sync.dma_start(
                    out=t_[C1:C1R, 0:L1], in_=onesrow_bf[0:1, 0:L1]
                )   # ones row for dx-fold
            for i, t_ in enumerate(dxreps):
                eng = nc.vector
                eng.memset(t_[0:C1R, L1:T], 0.0)
                nc.sync.dma_start(
                    out=t_[C1:C1R, 0:L1], in_=zrow_bf[0:1, 0:L1]
                )

            # all-batch tiles (column block per batch)
            h2all = spool.tile([128, NB * 128], F32, tag="h2all")
            h2sh = spool.tile([128, NB * 128], F32, tag="h2sh")
            d2f = spool.tile([128, NB * 128], F32, tag="d2f")
            d2 = spool.tile([128, NB * 128], F32R, tag="d2")
            m2p = spool.tile([128, NB * 128], F32R, tag="m2p")
            f2sb = spool.tile([H2, NB * H2], F32, tag="f2sb")
            f1sb = spool.tile([1, T], F32, tag="f1sb")
            nc.vector.memset(h2sh[0:1, :], 0.0)

            def stage_a(b):
                """augment1 conv stack + h assembly + m'/dx (pool)."""
                xsb = xpool.tile([20, T], F32R, tag="xsb")
                nc.sync.dma_start(out=xsb[:], in_=xs_d.ap()[b])

                r0 = r0s[b % 2]
                r1 = r1s[b % 2]
                h = hs[b % 2]
                for h0 in (0, 512):
                    ps0 = cv_ps.tile([H1, 512], F32, tag="cv")
                    mm(ps0[:], w0p[:], xsb[:, h0 : h0 + 512], True)
                    n = min(512, L1 - h0)
                    nc.scalar.activation(
                        r0[0:H1, h0 : h0 + n], ps0[:, 0:n], Relu, bias=b0c[:]
                    )
                for h0 in (0, 512):
                    ps1 = cv_ps.tile([H1, 512], F32, tag="cv")
                    mm(ps1[:], w1a[:], r0[0:H1, h0 : h0 + 512], True)
                    n = min(512, L1 - h0)
                    nc.scalar.activation(
                        r1[0:H1, h0 : h0 + n], ps1[:, 0:n], Relu, bias=b1c[:]
                    )
                for h0 in (0, 512):
                    ps2 = cv_ps.tile([H2, 512], F32, tag="cv")
                    mm(ps2[:], w2a[:], r1[:, h0 : h0 + 512], True)
                    n = min(512, L1 - h0)
                    nc.scalar.activation(
                        h[0:H2, h0 : h0 + n], ps2[:, 0:n], Copy
                    )

                # ---- assemble h (22, 1021): conv out, x[t+3,:], time ----
                nc.sync.dma_start(
                    out=h[16:21, 0:L1], in_=xs_d.ap()[b, 15:20, 0:L1].bitcast(F32)
                )

                # ---- m' = h[t]+h[t-1], dx = h[t]-h[t-1] (pool engine) ----
                mrep = mreps[b % 2]
                dxrep = dxreps[b % 2]
                nc.gpsimd.tensor_tensor(
                    mrep[0:C1, 1:L1], h[:, 1:L1], h[:, 0 : L1 - 1], ADD
                )
                nc.gpsimd.tensor_tensor(
                    dxrep[0:C1, 1:L1], h[:, 1:L1], h[:, 0 : L1 - 1], SUB
                )
                nc.gpsimd.tensor_copy(mrep[0:C1, 0:1], h[:, 0:1])
                nc.gpsimd.tensor_copy(dxrep[0:C1, 0:1], h[:, 0:1])

            def sig2_prep(b, c0, c1):
                """shift/diff/mask/sum of the h2 block cols [c0:c1)."""
                col = 128 * b
                nc.sync.dma_start(
                    out=h2sh[1:128, col + c0 : col + c1],
                    in_=h2all[0:127, col + c0 : col + c1],
                )
                j0, j1 = c0 // 16, c1 // 16
                jlo = max(j0, 1)
                sh_view = h2sh[0:1, col + 16 * jlo : col + 16 * j1].rearrange(
                    "p (j c) -> p j c", j=j1 - jlo
                )
                src_view = h2all[
                    127:128, col + 16 * (jlo - 1) : col + 16 * (j1 - 1)
                ].rearrange("p (j c) -> p j c", j=j1 - jlo)
                nc.sync.dma_start(out=sh_view, in_=src_view)

                eng_s = nc.vector if b == NB - 1 else nc.gpsimd
                eng_s.tensor_tensor(
                    d2f[:, col + c0 : col + c1],
                    h2all[:, col + c0 : col + c1],
                    h2sh[:, col + c0 : col + c1],
                    SUB,
                )
                mview = maskt[:, 8 * b + j0 : 8 * b + j1].unsqueeze(2)
                eng_m = nc.vector if b == NB - 1 else nc.gpsimd
                eng_m.tensor_tensor(
                    d2[:, col + c0 : col + c1].rearrange(
                        "p (j c) -> p j c", j=j1 - j0
                    ),
                    d2f[:, col + c0 : col + c1].rearrange(
                        "p (j c) -> p j c", j=j1 - j0
                    ),
                    mview.broadcast_to((128, j1 - j0, H2)),
                    MUL,
                )
                eng_s.tensor_tensor(
                    m2p[:, col + c0 : col + c1],
                    h2all[:, col + c0 : col + c1],
                    h2sh[:, col + c0 : col + c1],
                    ADD,
                )

            def stage_c_pe(b):
                """signature2 reductions for batch b (PE + tiny copies)."""
                col = 128 * b
                psF = cv_ps.tile([H2, H2], F32, tag="cv")
                for j in range(8):
                    cj = col + 16 * j
                    mm(
                        psF[:],
                        m2p[:, cj : cj + 16],
                        d2[:, cj : cj + 16],
                        j == 0,
                        j == 7,
                    )
                nc.scalar.activation(f2sb[:, H2 * b : H2 * (b + 1)], psF[:], Copy)
                psF1 = cv_ps.tile([1, 128], F32, tag="cv")
                mm(psF1[:], onescol[:].bitcast(F32R), d2[:, col : col + 128], True)
                nc.scalar.activation(f1sb[:, col : col + 128], psF1[:], Copy)

            def stage_b(b):
                """expansion, u matmuls, scan, y, augment2, h2, sig2 prep."""
                mrep = mreps[b % 2]
                dxrep = dxreps[b % 2]
                gbig = gbigs[b % 2]

                # ---- expansions + g = m'_exp * dx_exp (channel-major) ----
                mexp = gpool.tile([CW, 4 * T], BF16, tag="mexp")
                for phase in range(8):
                    h0 = 512 * (phase % 2)
                    c = phase // 2
                    mps = exp_ps.tile([CW, 512], F32, tag="exp")
                    mm(
                        mps[:],
                        ri[:, CW * c : CW * (c + 1)],
                        mrep[:, h0 : h0 + 512],
                        True,
                    )
                    mview = mexp[:, T * c + h0 : T * c + h0 + 512]
                    nc.scalar.activation(mview, mps[:], Copy)
                    dps = exp_ps.tile([CW, 512], F32, tag="exp")
                    mm(
                        dps[:],
                        rj[:, CW * c : CW * (c + 1)],
                        dxrep[:, h0 : h0 + 512],
                        True,
                    )
                    gview = gbig[:, T * c + h0 : T * c + h0 + 512]
                    nc.vector.tensor_tensor(gview, dps[:], mview, MUL)

                # ---- u0/u1 matmuls (fused 128-out, same rhs) ----
                psA = u_ps.tile([128, T], F32, tag="uA")   # rows 0:64 u0, 64:128 u1
                for h0 in (0, 512):
                    for c in range(4):
                        mm(
                            psA[:, h0 : h0 + 512],
                            v01[:, 128 * c : 128 * (c + 1)],
                            gbig[:, T * c + h0 : T * c + h0 + 512],
                            c == 0,
                            c == 3,
                        )
                # ---- cumsum(u0) + u1[t+1] ----
                scn = ypool.tile([H1, T], F32, tag="scn")
                nc.vector.tensor_tensor_scan(
                    scn[:, 0:L1], psA[0:H1, 0:L1], zeros[:, 0:L1], 0.0, ADD, ADD
                )
                t1 = ypool.tile([H1, T], F32, tag="t1")
                nc.vector.tensor_tensor(
                    t1[:, 0:L2], scn[:, 0:L2], psA[H1:128, 1 : 1 + L2], ADD
                )

                # ---- u2/u3 matmuls (fused 128-out, same rhs) ----
                psC = c_ps.tile([128, T], F32, tag="uC")   # rows 0:64 u2, 64:128 u3
                for h0 in (0, 512):
                    for c in range(4):
                        mm(
                            psC[:, h0 : h0 + 512],
                            v23[:, 128 * c : 128 * (c + 1)],
                            gbig[:, T * c + h0 : T * c + h0 + 512],
                            c == 0,
                            c == 3,
                        )

                # ---- y = relu(t1 + u2[t+2] + u3[t+3] + bias) ----
                y = ys[b % 2]
                t2 = ypool.tile([H1, T], F32, tag="t2")
                nc.vector.tensor_tensor(
                    t2[:, 0:L2], t1[:, 0:L2], psC[0:H1, 2 : 2 + L2], ADD
                )
                t3 = ypool.tile([H1, T], F32, tag="t3")
                nc.vector.tensor_tensor(
                    t3[:, 0:L2], t2[:, 0:L2], psC[H1:128, 3 : 3 + L2], ADD
                )
                nc.scalar.activation(
                    y[0:H1, 0:L2], t3[:, 0:L2], Relu, bias=ba0c[:]
                )

                # ---- fill PE with previous batch's sig2 reduction ----
                if b >= 1:
                    stage_c_pe(b - 1)

                # ---- augment2 pointwise convs ----
                r2 = r2s[b % 2]
                for h0 in (0, 512):
                    psY = cv_ps.tile([H1, 512], F32, tag="cv")
                    mm(psY[:], w1b[:], y[0:H1, h0 : h0 + 512], True)
                    n = min(512, L2 - h0)
                    nc.scalar.activation(
                        r2[0:H1, h0 : h0 + n], psY[:, 0:n], Relu, bias=b1bc[:]
                    )

                # conv2_2 data-stationary -> time-major h2 (t, ch) per block;
                # for the last batch, halve blocks so the sig2 chain pipelines
                col = 128 * b
                halves = ((0, 8),) if b < NB - 1 else ((0, 4), (4, 8))
                psH = cv_ps.tile([128, 128], F32, tag="cv")
                for (ja, jb) in halves:
                    for j in range(ja, jb):
                        mm(
                            psH[:, 16 * j : 16 * j + 16],
                            r2[:, 128 * j : 128 * j + 128],
                            w2b[:],
                            True,
                        )
                    nc.scalar.activation(
                        h2all[:, col + 16 * ja : col + 16 * jb],
                        psH[:, 16 * ja : 16 * jb],
                        Copy,
                    )
                    sig2_prep(b, 16 * ja, 16 * jb)

            # ---- head: presets done above; run the pipelined batch loop ----
            stage_a(0)

            # remaining constants (off the batch-0 critical path)
            ri = cload(ri_d, (C1R, NCH * CW))
            rj = cload(rj_d, (C1R, NCH * CW))
            v01 = cload(v01_d, (CW, 4 * 128))
            v23 = cload(v23_d, (CW, 4 * 128))
            ba0c = cload(ba0_d, (H1, 1))
            w1b = cload(w1b_d, (H1, H1))
            b1bc = cload(b1b_d, (H1, 1))
            w2b = cload(w2b_d, (H1 + 1, H2))
            maskt = cload(mask_d, (128, NB * 8))

            for b in range(NB):
                if b + 1 < NB:
                    stage_a(b + 1)
                stage_b(b)
            stage_c_pe(NB - 1)

            nc.sync.dma_start(out=f2o_d.ap(), in_=f2sb[:])
            nc.sync.dma_start(out=f1o_d.ap(), in_=f1sb[:])

    return nc


def _prep_host(x, lengths):
    """host-side preprocessing -> per-core input maps + host weights"""
    x = np.ascontiguousarray(x, dtype=np.float32)
    lengths = np.asarray(lengths).astype(np.int64)

    xs = np.zeros((B, 20, T), np.float32)
    for k in range(K):
        xs[:, 5 * k : 5 * k + 5, 0:L1] = x[:, k : k + L1, :].transpose(0, 2, 1)

    adj = (lengths - 2 * K + 2).astype(np.int64)  # (64,)
    # mask in time-major packed layout: mask[p, b*8+j] = 1 if (128j+p) < adj_b (and < L2)
    tgrid = (np.arange(8)[None, :] * 128 + np.arange(128)[:, None])  # (128, 8)
    masks = []
    for core in range(NCORES):
        mcols = np.zeros((128, NB * 8), np.float32)
        for b in range(NB):
            a = min(int(adj[core * NB + b]), L2)
            mcols[:, b * 8 : (b + 1) * 8] = (tgrid < a).astype(np.float32)
        masks.append(mcols)
    return xs, masks, adj


def round_f32r(a):
    """round-to-nearest-even to 11-bit mantissa (fp32r)"""
    u = np.ascontiguousarray(a, np.float32).view(np.uint32)
    u = (u + 0x7FF + ((u >> 12) & 1)) & np.uint32(0xFFFFF000)
    return u.view(np.float32)


F32R_KEYS = {"xs", "masktm", "w0p", "w1a", "w2a", "w1b", "w2b", "v01", "v23"}
BF16_KEYS = {"ri", "rj"}


def _prep_weights(inp):
    w = {}
    w["trow"] = np.zeros((1, T), np.float32)
    w["trow"][0, :L1] = np.linspace(0.0, 1.0, L1, dtype=np.float32)

    a1_w0 = inp["a1_w0"]
    w0p = np.zeros((20, H1), np.float32)
    for k in range(K):
        w0p[5 * k : 5 * k + 5, :] = a1_w0[:, :, k].T
    w["w0p"] = w0p
    w["b0c"] = inp["a1_b0"].reshape(H1, 1).astype(np.float32)
    w["w1a"] = inp["a1_w1"][:, :, 0].T.astype(np.float32)
    w["b1c"] = inp["a1_b1"].reshape(H1, 1).astype(np.float32)
    w2a = np.zeros((H1 + 1, H2), np.float32)
    w2a[0:H1] = inp["a1_w2"][:, :, 0].T
    w2a[H1] = inp["a1_b2"]
    w["w2a"] = w2a

    # selection matrices: 4 chunks of 128 cols; cols 0:121 replicate the g
    # rows, cols 121: carry raw dx channels (ones row in ri, channel in rj)
    ri = np.zeros((C1R, NCH * CW), np.float32)
    rj = np.zeros((C1R, NCH * CW), np.float32)
    for c in range(NCH):
        for p in range(CHW):
            g = CHW * c + p
            ri[g // C1, CW * c + p] = 1.0
            rj[g % C1, CW * c + p] = 1.0
        for e, ch in enumerate(EXC[c]):
            ri[C1, CW * c + CHW + e] = 1.0
            rj[ch, CW * c + CHW + e] = 1.0
    w["ri"] = ri
    w["rj"] = rj

    w20 = inp["a2_w0"]  # (64, 506, 4)
    # permute the a2 (484) block to the device h-channel order
    pidx = (np.array(PERM)[:, None] * C1 + np.array(PERM)[None, :]).reshape(-1)
    Wk_a = [w20[:, C1:, k].astype(np.float64)[:, pidx] for k in range(K)]
    V = [None] * 4
    V[3] = Wk_a[3]
    V[2] = Wk_a[2] + V[3]
    V[1] = Wk_a[1] + V[2]
    V[0] = Wk_a[0] + V[1]
    perm = PERM
    Wh = [w20[:, perm, k].T.astype(np.float64) for k in range(K)]  # (22, 64)
    Vh = [None] * 4
    Vh[3] = Wh[3]
    Vh[2] = Wh[2] + Vh[3]
    Vh[1] = Wh[1] + Vh[2]
    Vh[0] = Wh[0] + Vh[1]
    # halve the g part for m' = 2m; dx-fold rows are unhalved
    v01 = np.zeros((CW, 4 * 128), np.float32)
    v23 = np.zeros((CW, 4 * 128), np.float32)
    for c in range(NCH):
        rows = slice(CHW * c, CHW * (c + 1))
        v01[0:CHW, 128 * c : 128 * c + 64] = 0.5 * V[0].T[rows]
        v01[0:CHW, 128 * c + 64 : 128 * c + 128] = 0.5 * V[1].T[rows]
        v23[0:CHW, 128 * c : 128 * c + 64] = 0.5 * V[2].T[rows]
        v23[0:CHW, 128 * c + 64 : 128 * c + 128] = 0.5 * V[3].T[rows]
        for e, ch in enumerate(EXC[c]):
            v01[CHW + e, 128 * c : 128 * c + 64] = Vh[0][ch]
            v01[CHW + e, 128 * c + 64 : 128 * c + 128] = Vh[1][ch]
            v23[CHW + e, 128 * c : 128 * c + 64] = Vh[2][ch]
            v23[CHW + e, 128 * c + 64 : 128 * c + 128] = Vh[3][ch]
    w["v01"] = v01
    w["v23"] = v23

    w["ba0c"] = inp["a2_b0"].reshape(H1, 1).astype(np.float32)
    w["w1b"] = inp["a2_w1"][:, :, 0].T.astype(np.float32)
    w["b1bc"] = inp["a2_b1"].reshape(H1, 1).astype(np.float32)
    w2b = np.zeros((H1 + 1, H2), np.float32)
    w2b[0:H1] = inp["a2_w2"][:, :, 0].T
    w2b[H1] = inp["a2_b2"]
    w["w2b"] = w2b
    np_bf16 = mybir.dt.np(mybir.dt.bfloat16)
    for k in list(w):
        if k in F32R_KEYS:
            w[k] = round_f32r(w[k])
        elif k in BF16_KEYS:
            w[k] = w[k].astype(np_bf16)
    return w


def kernel(**inputs):
    global _COMPILED
    x = np.asarray(inputs["x"], np.float32)
    lengths = np.asarray(inputs["lengths"])

    xs, masks, adj = _prep_host(x, lengths)
    w = _prep_weights({k: np.asarray(v) for k, v in inputs.items()})

    if _COMPILED is None:
        _c = build_program()
        _c.finalize()
        _COMPILED = _c
    nc = _COMPILED

    in_maps = []
    for core in range(NCORES):
        m = {"xs": round_f32r(xs[core * NB : (core + 1) * NB]),
             "masktm": masks[core]}
        m.update(w)
        in_maps.append(m)

    _res = run_bass_kernel_spmd(nc, in_maps, list(range(NCORES)), trace=TRACE)
    globals()["LAST"] = _res
    res = _res.results

    # host: assemble s2 and final linear
    lin_w = np.asarray(inputs["lin_w"], np.float32)
    lin_b = np.asarray(inputs["lin_b"], np.float32)
    out = np.zeros((B, OUT), np.float32)
    for core in range(NCORES):
        f2 = res[core]["f2o"]          # (16, NB*16)
        f1 = res[core]["f1o"][0]       # (T,)
        for b in range(NB):
            gb = core * NB + b
            F2 = 0.5 * f2[:, H2 * b : H2 * (b + 1)]          # (16, 16)
            F1 = f1[128 * b : 128 * (b + 1)].reshape(8, H2).sum(axis=0)
            s2 = np.concatenate([F1, F2.reshape(-1)])
            out[gb] = s2 @ lin_w.T + lin_b
    return out.astype(np.float32)
